# revision 24
# baseline (speedup 1.0000x reference)
"""Trainium2 Bass kernel for nn_NeuralNetworkSimplified (binarized 4-layer MLP + BN).

Math
----
reference computes, per hidden layer l (gamma=1, beta=0, biases b_l arbitrary):
    z = sign(a) @ sign(W).T + sign(b)
    h = clip(batchnorm_train(z), -1, 1)
and the next layer only consumes sign(h).  Since batchnorm's rsqrt(var+eps) > 0
and gamma=1/beta=0, sign(h) = sign(z - mean_batch(z)); the sign(b) bias shifts
z and its mean equally, so it cancels.  The whole network therefore reduces to
exact integer arithmetic:
    S0 = sign(x)
    S_l = sign(Z_l - colmean(Z_l)),   Z_l = S_{l-1} @ sign(W_l).T
    out = S3 @ sign(W4).T + sign(b4)

On device activations are +-0.5 (fp8) and weights +-1 (fp8), so Z comes out as
exact half-scaled integer dot products (|Z| <= 1536: exact in fp32 psum and in
the fp16 spill).  The batch mean needs a cross-core reduction: each core
computes the column sum of its local Z block as a fused by-product of the
PSUM->SBUF spill on the Scalar engine (activation Copy with accum_out), then
AllReduces the tiny [128, 8] partial-sum tile per pair of output blocks.
Binarize (DVE) reads the fp16 Z copy, so PSUM recycles immediately and the
AllReduce latency never blocks the PE.

Sharding: batch 16384 -> 8 cores x 2048.  Activations live feature-major
(transposed) so the contraction dim sits on SBUF partitions; host passes x.T
shards and W_l.T (layout-only prep; all FLOPs incl. sign() run on device).

Engines: PE runs only the z-matmuls (fp8 DoubleRow).  Scalar does all weight
binarize (Sign) + Z spills + final bias.  DVE/GpSimd split the x binarize; DVE
does S binarize.  DMA of x/W1 is interleaved kp-by-kp so layer 1 streams
right behind the load.
"""

import numpy as np
import ml_dtypes

B, D, H1, H2, H3, C = 16384, 3072, 2048, 2048, 1024, 512
NCORES = 8
BL = B // NCORES          # 2048 rows per core
NF = 512                  # batch free-dim chunk (psum tile width)
NCH = BL // NF            # 4 chunks
LAYERS = [(D, H1), (H1, H2), (H2, H3), (H3, C)]

_CACHE = {}


def _build_module():
    import concourse.bass as bass
    import concourse.mybir as mybir
    import concourse.tile as tile
    from concourse import bacc

    mdt = mybir.dt
    FP8 = mdt.float8e4
    ALU = mybir.AluOpType
    ACT = mybir.ActivationFunctionType
    DR = mybir.MatmulPerfMode.DoubleRow

    nc = bacc.Bacc(
        "TRN2",
        target_bir_lowering=False,
        debug=False,
        num_devices=NCORES,
    )

    xT = nc.dram_tensor("xT", [D, BL], mdt.bfloat16, kind="ExternalInput").ap()
    wT = [
        nc.dram_tensor(f"w{i + 1}t", [K, H], mdt.bfloat16, kind="ExternalInput").ap()
        for i, (K, H) in enumerate(LAYERS)
    ]
    b4 = nc.dram_tensor("b4", [C, 1], mdt.float32, kind="ExternalInput").ap()
    outT = nc.dram_tensor("outT", [C, BL], mdt.float16, kind="ExternalOutput").ap()

    # t AllReduce buffers: one per (hidden layer, 2-block group); each carries
    # 4 chunk-partials x 2 blocks = [128, 8] fp32.
    n_groups = [H1 // 256, H2 // 256, H3 // 256]   # 8, 8, 4
    cc_in = [
        [nc.dram_tensor(f"tin{l}_{g}", [128, 8], mdt.float32).ap()
         for g in range(n_groups[l])]
        for l in range(3)
    ]
    cc_out = [
        [nc.dram_tensor(f"tout{l}_{g}", [128, 8], mdt.float32,
                        addr_space="Shared").ap()
         for g in range(n_groups[l])]
        for l in range(3)
    ]

    with tile.TileContext(nc, num_cores=NCORES) as tc:
        with (
            tc.tile_pool(name="rawx", bufs=2) as rawx,     # x staging pair tiles
            tc.tile_pool(name="raww", bufs=2) as raww,     # W staging pair tiles
            tc.tile_pool(name="sA", bufs=12) as sA,        # S0, S2 pair tiles
            tc.tile_pool(name="sB", bufs=8) as sB,         # S1, S3 pair tiles
            tc.tile_pool(name="wA", bufs=12) as wA,        # W1~, W3~ pair tiles
            tc.tile_pool(name="wB", bufs=8) as wB,         # W2~, W4~ pair tiles
            tc.tile_pool(name="zf", bufs=5) as zf,         # fp16 Z spill blocks
            tc.tile_pool(name="stat", bufs=2) as stat,     # partial sums etc.
            tc.tile_pool(name="ou", bufs=4) as ou,         # fp16 output chunks
            tc.tile_pool(name="pz", bufs=8, space="PSUM") as pz,
        ):
            # ---- PE warm-up: dummy matmuls so HAM unthrottles while the
            # binarize prologue runs on the other engines.
            warm = stat.tile([128, 128], FP8, tag="warm")
            nc.vector.memset(warm, 0.5)
            wps = pz.tile([128, NF], mdt.float32, tag="pz", name="warmps")
            for i in range(24):
                nc.tensor.matmul(wps[:, 0:128], warm, warm, start=True, stop=True)

            # ---- sign(b4) as +-1 per-partition vector, [128, C//128] ----
            b4_sb = stat.tile([128, C // 128], mdt.float32, tag="rawb")
            nc.sync.dma_start(b4_sb, b4.rearrange("(o p) q -> p (o q)", p=128))
            sb4 = stat.tile([128, C // 128], mdt.float32, tag="sb4")
            nc.vector.tensor_scalar(
                out=sb4, in0=b4_sb, scalar1=0.0, scalar2=2.0,
                op0=ALU.is_ge, op1=ALU.mult,
            )
            nc.vector.tensor_scalar_add(sb4, sb4, -1.0)

            def bin_w(w8, rt):
                # sign(W)/2 as +-0.5 fp8 on DVE.  The two scalar literals
                # (0.0, 0.5) are exact in every intermediate precision, so
                # this is immune to the tensor_scalar scalar-degrade issue.
                nc.vector.tensor_scalar(
                    out=w8, in0=rt, scalar1=0.0, scalar2=0.5,
                    op0=ALU.is_ge, op1=ALU.subtract,
                )

            def dma_w_tile(pool, idx, kp, dma_eng=None):
                # DMA the two k-halves of a weight pair tile; binarize is a
                # deferred list of per-half (dst, src) DVE ops.
                K, H = LAYERS[idx]
                w8 = pool.tile([128, 2, H], FP8, tag="w", name=f"w{idx}_{kp}")
                halves = []
                for h in range(2):
                    rt = raww.tile([128, H], mdt.bfloat16, tag="raww",
                                   name=f"rw{idx}_{kp}_{h}")
                    (dma_eng or nc.sync).dma_start(
                        rt,
                        wT[idx][(2 * kp + h) * 128:(2 * kp + h + 1) * 128, :],
                    )
                    halves.append((w8[:, h, :], rt))
                return halves, w8

            # ---- x + W1 prologue, interleaved kp-by-kp on the sync queue so
            # layer 1's contraction streams right behind the load.
            S0 = []
            W8_1 = []
            for kp in range(D // 256):
                s8 = sA.tile([128, 2, BL], FP8, tag="s", name=f"s0_{kp}")
                for h in range(2):
                    rx = rawx.tile([128, BL], mdt.bfloat16, tag="rawx",
                                   name=f"rx{kp}_{h}")
                    nc.sync.dma_start(
                        rx,
                        xT[(2 * kp + h) * 128:(2 * kp + h + 1) * 128, :],
                    )
                    # sign(x) as +-1 fp8 via ACT (runs parallel to the DVE
                    # weight binarize during the DMA-bound startup).  x is
                    # never exactly +-0.0 (bf16 keeps fp32's exponent range),
                    # so Sign(0)=0 cannot occur.
                    nc.scalar.activation(out=s8[:, h, :], in_=rx, func=ACT.Sign)
                S0.append(s8)
                # W1 stream rides the gpsimd SWDGE queue so it drains in
                # parallel with the x stream on the sync queue (each queue
                # alone tops out ~230 GB/s; together they reach the HBM cap).
                halves, w8 = dma_w_tile(wA, 0, kp, dma_eng=nc.gpsimd)
                for dst, src in halves:
                    bin_w(dst, src)
                W8_1.append(w8)

            # W2 raw DMAs issue now (sync queue, after x/W1); binarize is
            # deferred into the layer-1 m-loop so Scalar prioritizes spills.
            W2_def = []
            W8_2 = []
            for kp in range(H1 // 256):
                halves, w8 = dma_w_tile(wB, 1, kp)
                W2_def.extend(halves)
                W8_2.append(w8)

            def alloc_s(pool, H, nm):
                return [
                    pool.tile([128, 2, BL], FP8, tag="s", name=f"{nm}_{i}")
                    for i in range(H // 256)
                ]

            # ---- shared matmul emission for a 2-block group ----
            # For g == 0 the two blocks' contraction loops are interleaved
            # (all-but-last kp for both, then last kp for both): the last kp
            # consumes the previous layer's final S pair, which waits on that
            # layer's last AllReduce, so this buys ~2 blocks of PE cover.
            # Also streams layer 1 behind the x/W1 DMA arrival order.
            def group_mms(W8, A8, KT, psA, psB, mcA, mcB, interleave):
                KH = KT // 2

                def mm(ps, mc, kp):
                    st, sp = kp == 0, kp == KH - 1
                    wsl = W8[kp][:, :, mc]
                    for n in range(NCH):
                        nc.tensor.matmul(
                            ps[n], wsl, A8[kp][:, :, n * NF:(n + 1) * NF],
                            start=st, stop=sp, perf_mode=DR,
                        )

                if interleave:
                    for kp in range(KH - 1):
                        mm(psA, mcA, kp)
                    for kp in range(KH - 1):
                        mm(psB, mcB, kp)
                    mm(psA, mcA, KH - 1)
                    mm(psB, mcB, KH - 1)
                else:
                    for kp in range(KH):
                        mm(psA, mcA, kp)
                    for kp in range(KH):
                        mm(psB, mcB, kp)

            # ---- one hidden layer ----
            # Per output block m: z-matmuls -> 4 psum chunks; Scalar spills
            # each chunk into the fp16 Z tile with a fused column-sum partial;
            # per 2-block group AllReduce the [128, 8] partials; DVE reduces
            # the 4 global partials, scales by 1/B, binarizes Z vs threshold.
            def hidden_layer(l, A8, W8, S_out, deferred):
                K, H = LAYERS[l]
                KT, MT = K // 128, H // 128
                for g in range(MT // 2):
                    ma, mb = 2 * g, 2 * g + 1
                    mcA = slice(ma * 128, (ma + 1) * 128)
                    mcB = slice(mb * 128, (mb + 1) * 128)
                    tpart = stat.tile([128, 8], mdt.float32, tag="tpart",
                                      bufs=4, name=f"tp{l}_{g}")
                    zA = zf.tile([128, BL], mdt.float16, tag="z",
                                 name=f"z{l}_{ma}")
                    zB = zf.tile([128, BL], mdt.float16, tag="z",
                                 name=f"z{l}_{mb}")
                    psA = [pz.tile([128, NF], mdt.float32, tag="pz",
                                   name=f"pz{l}_{ma}_{n}") for n in range(NCH)]
                    psB = [pz.tile([128, NF], mdt.float32, tag="pz",
                                   name=f"pz{l}_{mb}_{n}") for n in range(NCH)]
                    group_mms(W8, A8, KT, psA, psB, mcA, mcB, g == 0)
                    for j, (z16, ps) in enumerate(((zA, psA), (zB, psB))):
                        for n in range(NCH):
                            nc.scalar.activation(
                                out=z16[:, n * NF:(n + 1) * NF], in_=ps[n],
                                func=ACT.Copy,
                                accum_out=tpart[:, 4 * j + n:4 * j + n + 1],
                            )
                    nc.gpsimd.dma_start(cc_in[l][g][:, :], tpart)
                    nc.gpsimd.collective_compute(
                        "AllReduce",
                        ALU.add,
                        replica_groups=[list(range(NCORES))],
                        ins=[cc_in[l][g][:, :]],
                        outs=[cc_out[l][g][:, :]],
                    )
                    tg = stat.tile([128, 8], mdt.float32, tag="tg",
                                   bufs=3, name=f"tg{l}_{g}")
                    nc.gpsimd.dma_start(tg, cc_out[l][g][:, :])
                    thr = stat.tile([128, 2], mdt.float32, tag="thr",
                                    bufs=3, name=f"th{l}_{g}")
                    for j in range(2):
                        nc.vector.tensor_reduce(
                            thr[:, j:j + 1], tg[:, 4 * j:4 * j + 4],
                            mybir.AxisListType.X, ALU.add,
                        )
                    nc.vector.tensor_scalar_mul(thr, thr, 1.0 / B)
                    # S = (z >= mean) as {0,1}: BN's mean subtraction cancels
                    # any per-row affine encoding offset downstream, so {0,1}
                    # is as good as +-1 for hidden activations.  tensor_tensor
                    # keeps the fp32 threshold exact (tensor_scalar degrades
                    # AP scalars against 16-bit inputs on HW).
                    for j, z16 in enumerate((zA, zB)):
                        mm = 2 * g + j
                        nc.vector.tensor_tensor(
                            out=S_out[mm // 2][:, mm % 2, :],
                            in0=z16,
                            in1=thr[:, j:j + 1].broadcast_to([128, BL]),
                            op=ALU.is_ge,
                        )
                    # deferred weight binarize after the AR-dependent DVE ops
                    # so a layer-boundary binarize never queues behind them
                    for _ in range(2):
                        if deferred:
                            bin_w(*deferred.pop(0))

            # ---- final layer ----
            # S3 is {0,1}-encoded: psum = sum_k a3*w4~ = (d4 + R4_m)/4 with
            # R4_m = sum_k sign(W4[m,k]).  The true output is
            # out = d4 + sign(b4) = 4*psum - R4_m + sign(b4); R4 = 2*rowsum(w4~)
            # comes from 16 tiny DoubleRow matmuls against a ones vector.
            def final_layer(A8, W8):
                K, H = LAYERS[3]
                KT, MT = K // 128, H // 128
                ones8 = stat.tile([128, 2, 1], FP8, tag="ones")
                nc.vector.memset(ones8, 1.0)
                rps = pz.tile([128, NF], mdt.float32, tag="pz", name="rps")
                for m in range(MT):
                    mc = slice(m * 128, (m + 1) * 128)
                    for kp in range(KT // 2):
                        nc.tensor.matmul(
                            rps[:, m:m + 1], W8[kp][:, :, mc], ones8,
                            start=(kp == 0), stop=(kp == KT // 2 - 1),
                            perf_mode=DR,
                        )
                bias4 = stat.tile([128, MT], mdt.float32, tag="bias4")
                nc.vector.tensor_single_scalar(
                    out=bias4, in_=rps[:, 0:MT], scalar=-2.0, op=ALU.mult,
                )
                nc.vector.tensor_add(bias4, bias4, sb4)
                for g in range(MT // 2):
                    ma, mb = 2 * g, 2 * g + 1
                    mcA = slice(ma * 128, (ma + 1) * 128)
                    mcB = slice(mb * 128, (mb + 1) * 128)
                    psA = [pz.tile([128, NF], mdt.float32, tag="pz",
                                   name=f"pz3_{ma}_{n}") for n in range(NCH)]
                    psB = [pz.tile([128, NF], mdt.float32, tag="pz",
                                   name=f"pz3_{mb}_{n}") for n in range(NCH)]
                    group_mms(W8, A8, KT, psA, psB, mcA, mcB, True)
                    for (ps, m, mc) in ((psA, ma, mcA), (psB, mb, mcB)):
                        for n in range(NCH):
                            oc = ou.tile([128, NF], mdt.float16, tag="ot",
                                         bufs=4, name=f"ot{m}_{n}")
                            nc.scalar.activation(
                                out=oc, in_=ps[n],
                                func=ACT.Identity, bias=bias4[:, m:m + 1],
                                scale=4.0,
                            )
                            nc.sync.dma_start(
                                outT[mc, n * NF:(n + 1) * NF], oc,
                            )

            # layer 1 (W2 binarize interleaved)
            S1 = alloc_s(sB, H1, "s1")
            hidden_layer(0, S0, W8_1, S1, W2_def)

            # layer 2 (W3 prep DMA + deferred binarize interleaved)
            W3_def = []
            W8_3 = []
            for kp in range(H2 // 256):
                halves, w8 = dma_w_tile(wA, 2, kp)
                W3_def.extend(halves)
                W8_3.append(w8)
            S2 = alloc_s(sA, H2, "s2")
            hidden_layer(1, S1, W8_2, S2, W3_def)

            # layer 3 (W4 prep interleaved)
            W4_def = []
            W8_4 = []
            for kp in range(H3 // 256):
                halves, w8 = dma_w_tile(wB, 3, kp)
                W4_def.extend(halves)
                W8_4.append(w8)
            S3 = alloc_s(sB, H3, "s3")
            hidden_layer(2, S2, W8_3, S3, W4_def)

            # layer 4
            final_layer(S3, W8_4)

    nc.compile()
    return nc


def _get_module():
    if "nc" not in _CACHE:
        _CACHE["nc"] = _build_module()
    return _CACHE["nc"]


def _reference_fallback(x, W1, b1, g1, be1, W2, b2, g2, be2, W3, b3, g3, be3, W4, b4):
    """Exact numpy clone of the reference for non-trivial gamma/beta inputs."""
    EPS = 1e-5

    def binarize(v):
        return np.where(v >= 0, 1.0, -1.0).astype(np.float32)

    def bin_linear(a, W, b):
        return binarize(a) @ binarize(W).T + binarize(b)

    def bn(z, g, be):
        m = z.mean(axis=0)
        v = z.var(axis=0)
        return (z - m) / np.sqrt(v + EPS) * g + be

    h = np.clip(bn(bin_linear(x, W1, b1), g1, be1), -1.0, 1.0)
    h = np.clip(bn(bin_linear(h, W2, b2), g2, be2), -1.0, 1.0)
    h = np.clip(bn(bin_linear(h, W3, b3), g3, be3), -1.0, 1.0)
    return bin_linear(h, W4, b4).astype(np.float32)


def make_in_maps(inputs):
    bf16 = ml_dtypes.bfloat16
    x = inputs["x"]
    common = {
        "w1t": np.ascontiguousarray(np.asarray(inputs["W1"]).T).astype(bf16),
        "w2t": np.ascontiguousarray(np.asarray(inputs["W2"]).T).astype(bf16),
        "w3t": np.ascontiguousarray(np.asarray(inputs["W3"]).T).astype(bf16),
        "w4t": np.ascontiguousarray(np.asarray(inputs["W4"]).T).astype(bf16),
        "b4": np.asarray(inputs["b4"], dtype=np.float32).reshape(C, 1),
    }
    in_maps = []
    for c in range(NCORES):
        m = dict(common)
        m["xT"] = np.ascontiguousarray(
            np.asarray(x[c * BL:(c + 1) * BL, :]).T
        ).astype(bf16)
        in_maps.append(m)
    return in_maps


def gather_output(results):
    out = np.empty((B, C), dtype=np.float32)
    for c in range(NCORES):
        out[c * BL:(c + 1) * BL, :] = results[c]["outT"].T.astype(np.float32)
    return out


def kernel(**inputs):
    # BN gamma/beta must be trivial for the sign-reduction; spec fills guarantee
    # this (g=ones, be=zeros).  Anything else falls back to exact host compute.
    for gk, bek in (("g1", "be1"), ("g2", "be2"), ("g3", "be3")):
        if not (np.all(np.asarray(inputs[gk]) == 1.0)
                and np.all(np.asarray(inputs[bek]) == 0.0)):
            return _reference_fallback(**{
                k: np.asarray(v, dtype=np.float32) for k, v in inputs.items()
            })

    from concourse.bass_utils import run_bass_kernel_spmd

    nc = _get_module()
    in_maps = make_in_maps(inputs)
    res = run_bass_kernel_spmd(nc, in_maps, list(range(NCORES)))
    return gather_output(res.results)


if __name__ == "__main__":
    nc = _get_module()
    print("module built OK")


# revision 29
# speedup vs baseline: 1.0182x; 1.0182x over previous
"""Trainium2 Bass kernel for nn_NeuralNetworkSimplified (binarized 4-layer MLP + BN).

Math
----
reference computes, per hidden layer l (gamma=1, beta=0, biases b_l arbitrary):
    z = sign(a) @ sign(W).T + sign(b)
    h = clip(batchnorm_train(z), -1, 1)
and the next layer only consumes sign(h).  Since batchnorm's rsqrt(var+eps) > 0
and gamma=1/beta=0, sign(h) = sign(z - mean_batch(z)); the sign(b) bias shifts
z and its mean equally, so it cancels.  The whole network therefore reduces to
exact integer arithmetic:
    S0 = sign(x)
    S_l = sign(Z_l - colmean(Z_l)),   Z_l = S_{l-1} @ sign(W_l).T
    out = S3 @ sign(W4).T + sign(b4)

On device activations are +-0.5 (fp8) and weights +-1 (fp8), so Z comes out as
exact half-scaled integer dot products (|Z| <= 1536: exact in fp32 psum and in
the fp16 spill).  The batch mean needs a cross-core reduction: each core
computes the column sum of its local Z block as a fused by-product of the
PSUM->SBUF spill on the Scalar engine (activation Copy with accum_out), then
AllReduces the tiny [128, 8] partial-sum tile per pair of output blocks.
Binarize (DVE) reads the fp16 Z copy, so PSUM recycles immediately and the
AllReduce latency never blocks the PE.

Sharding: batch 16384 -> 8 cores x 2048.  Activations live feature-major
(transposed) so the contraction dim sits on SBUF partitions; host passes x.T
shards and W_l.T (layout-only prep; all FLOPs incl. sign() run on device).

Engines: PE runs only the z-matmuls (fp8 DoubleRow).  Scalar does all weight
binarize (Sign) + Z spills + final bias.  DVE/GpSimd split the x binarize; DVE
does S binarize.  DMA of x/W1 is interleaved kp-by-kp so layer 1 streams
right behind the load.
"""

import numpy as np
import ml_dtypes

B, D, H1, H2, H3, C = 16384, 3072, 2048, 2048, 1024, 512
NCORES = 8
BL = B // NCORES          # 2048 rows per core
NF = 512                  # batch free-dim chunk (psum tile width)
NCH = BL // NF            # 4 chunks
LAYERS = [(D, H1), (H1, H2), (H2, H3), (H3, C)]

_CACHE = {}


def _build_module():
    import concourse.bass as bass
    import concourse.mybir as mybir
    import concourse.tile as tile
    from concourse import bacc

    mdt = mybir.dt
    FP8 = mdt.float8e4
    ALU = mybir.AluOpType
    ACT = mybir.ActivationFunctionType
    DR = mybir.MatmulPerfMode.DoubleRow

    nc = bacc.Bacc(
        "TRN2",
        target_bir_lowering=False,
        debug=False,
        num_devices=NCORES,
    )

    xT = nc.dram_tensor("xT", [D, BL], mdt.bfloat16, kind="ExternalInput").ap()
    wT = [
        nc.dram_tensor(f"w{i + 1}t", [K, H], mdt.bfloat16, kind="ExternalInput").ap()
        for i, (K, H) in enumerate(LAYERS)
    ]
    b4 = nc.dram_tensor("b4", [C, 1], mdt.float32, kind="ExternalInput").ap()
    outT = nc.dram_tensor("outT", [C, BL], mdt.float16, kind="ExternalOutput").ap()

    # t AllReduce buffers: one per (hidden layer, 2-block group); each carries
    # 4 chunk-partials x 2 blocks = [128, 8] fp32.
    n_groups = [H1 // 256, H2 // 256, H3 // 256]   # 8, 8, 4
    cc_in = [
        [nc.dram_tensor(f"tin{l}_{g}", [128, 8], mdt.float32).ap()
         for g in range(n_groups[l])]
        for l in range(3)
    ]
    cc_out = [
        [nc.dram_tensor(f"tout{l}_{g}", [128, 8], mdt.float32,
                        addr_space="Shared").ap()
         for g in range(n_groups[l])]
        for l in range(3)
    ]

    with tile.TileContext(nc, num_cores=NCORES) as tc:
        with (
            tc.tile_pool(name="rawx", bufs=2) as rawx,     # x staging pair tiles
            tc.tile_pool(name="raww", bufs=2) as raww,     # W staging pair tiles
            tc.tile_pool(name="sA", bufs=12) as sA,        # S0, S2 pair tiles
            tc.tile_pool(name="sB", bufs=8) as sB,         # S1, S3 pair tiles
            tc.tile_pool(name="wA", bufs=12) as wA,        # W1~, W3~ pair tiles
            tc.tile_pool(name="wB", bufs=8) as wB,         # W2~, W4~ pair tiles
            tc.tile_pool(name="zf", bufs=5) as zf,         # fp16 Z spill blocks
            tc.tile_pool(name="stat", bufs=2) as stat,     # partial sums etc.
            tc.tile_pool(name="ou", bufs=4) as ou,         # fp16 output chunks
            tc.tile_pool(name="pz", bufs=8, space="PSUM") as pz,
        ):
            # ---- PE warm-up: dummy matmuls so HAM unthrottles while the
            # binarize prologue runs on the other engines.
            warm = stat.tile([128, 128], FP8, tag="warm")
            nc.vector.memset(warm, 0.5)
            wps = pz.tile([128, NF], mdt.float32, tag="pz", name="warmps")
            for i in range(24):
                nc.tensor.matmul(wps[:, 0:128], warm, warm, start=True, stop=True)

            # ---- sign(b4) as +-1 per-partition vector, [128, C//128] ----
            b4_sb = stat.tile([128, C // 128], mdt.float32, tag="rawb")
            nc.sync.dma_start(b4_sb, b4.rearrange("(o p) q -> p (o q)", p=128))
            sb4 = stat.tile([128, C // 128], mdt.float32, tag="sb4")
            nc.vector.tensor_scalar(
                out=sb4, in0=b4_sb, scalar1=0.0, scalar2=2.0,
                op0=ALU.is_ge, op1=ALU.mult,
            )
            nc.vector.tensor_scalar_add(sb4, sb4, -1.0)

            def bin_w(w8, rt):
                # sign(W)/2 as +-0.5 fp8 on DVE.  The two scalar literals
                # (0.0, 0.5) are exact in every intermediate precision, so
                # this is immune to the tensor_scalar scalar-degrade issue.
                nc.vector.tensor_scalar(
                    out=w8, in0=rt, scalar1=0.0, scalar2=0.5,
                    op0=ALU.is_ge, op1=ALU.subtract,
                )

            def dma_w_tile(pool, idx, kp, dma_engs=None):
                # DMA the two k-halves of a weight pair tile; binarize is a
                # deferred list of per-half (dst, src) DVE ops.
                K, H = LAYERS[idx]
                w8 = pool.tile([128, 2, H], FP8, tag="w", name=f"w{idx}_{kp}")
                halves = []
                for h in range(2):
                    rt = raww.tile([128, H], mdt.bfloat16, tag="raww",
                                   name=f"rw{idx}_{kp}_{h}")
                    eng = dma_engs[h] if dma_engs else nc.sync
                    eng.dma_start(
                        rt,
                        wT[idx][(2 * kp + h) * 128:(2 * kp + h + 1) * 128, :],
                    )
                    halves.append((w8[:, h, :], rt))
                return halves, w8

            # ---- x + W1 prologue, interleaved kp-by-kp on the sync queue so
            # layer 1's contraction streams right behind the load.
            S0 = []
            W8_1 = []
            for kp in range(D // 256):
                s8 = sA.tile([128, 2, BL], FP8, tag="s", name=f"s0_{kp}")
                for h in range(2):
                    rx = rawx.tile([128, BL], mdt.bfloat16, tag="rawx",
                                   name=f"rx{kp}_{h}")
                    # split the stream over the two HWDGE queues (sync + ACT):
                    # one queue serializes 512KB transfers at ~240 GB/s; two
                    # run at the HBM cap.
                    (nc.sync if h == 0 else nc.scalar).dma_start(
                        rx,
                        xT[(2 * kp + h) * 128:(2 * kp + h + 1) * 128, :],
                    )
                    # sign(x) as +-1 fp8 via ACT (runs parallel to the DVE
                    # weight binarize during the DMA-bound startup).  x is
                    # never exactly +-0.0 (bf16 keeps fp32's exponent range),
                    # so Sign(0)=0 cannot occur.
                    nc.scalar.activation(out=s8[:, h, :], in_=rx, func=ACT.Sign)
                S0.append(s8)
                halves, w8 = dma_w_tile(wA, 0, kp,
                                        dma_engs=(nc.sync, nc.scalar))
                for dst, src in halves:
                    bin_w(dst, src)
                W8_1.append(w8)

            # W2 raw DMAs issue now (sync queue, after x/W1); binarize is
            # deferred into the layer-1 m-loop so Scalar prioritizes spills.
            W2_def = []
            W8_2 = []
            for kp in range(H1 // 256):
                halves, w8 = dma_w_tile(wB, 1, kp)
                W2_def.extend(halves)
                W8_2.append(w8)

            def alloc_s(pool, H, nm):
                return [
                    pool.tile([128, 2, BL], FP8, tag="s", name=f"{nm}_{i}")
                    for i in range(H // 256)
                ]

            # ---- shared matmul emission for a 2-block group ----
            # For g == 0 the two blocks' contraction loops are interleaved
            # (all-but-last kp for both, then last kp for both): the last kp
            # consumes the previous layer's final S pair, which waits on that
            # layer's last AllReduce, so this buys ~2 blocks of PE cover.
            # Also streams layer 1 behind the x/W1 DMA arrival order.
            def group_mms(W8, A8, KT, psA, psB, mcA, mcB, interleave):
                KH = KT // 2

                def mm(ps, mc, kp):
                    st, sp = kp == 0, kp == KH - 1
                    wsl = W8[kp][:, :, mc]
                    for n in range(NCH):
                        nc.tensor.matmul(
                            ps[n], wsl, A8[kp][:, :, n * NF:(n + 1) * NF],
                            start=st, stop=sp, perf_mode=DR,
                        )

                if interleave:
                    for kp in range(KH - 1):
                        mm(psA, mcA, kp)
                    for kp in range(KH - 1):
                        mm(psB, mcB, kp)
                    mm(psA, mcA, KH - 1)
                    mm(psB, mcB, KH - 1)
                else:
                    for kp in range(KH):
                        mm(psA, mcA, kp)
                    for kp in range(KH):
                        mm(psB, mcB, kp)

            # ---- one hidden layer ----
            # Per output block m: z-matmuls -> 4 psum chunks; Scalar spills
            # each chunk into the fp16 Z tile with a fused column-sum partial;
            # per 2-block group AllReduce the [128, 8] partials; DVE reduces
            # the 4 global partials, scales by 1/B, binarizes Z vs threshold.
            def hidden_layer(l, A8, W8, S_out, deferred):
                K, H = LAYERS[l]
                KT, MT = K // 128, H // 128
                for g in range(MT // 2):
                    ma, mb = 2 * g, 2 * g + 1
                    mcA = slice(ma * 128, (ma + 1) * 128)
                    mcB = slice(mb * 128, (mb + 1) * 128)
                    tpart = stat.tile([128, 8], mdt.float32, tag="tpart",
                                      bufs=4, name=f"tp{l}_{g}")
                    zA = zf.tile([128, BL], mdt.float16, tag="z",
                                 name=f"z{l}_{ma}")
                    zB = zf.tile([128, BL], mdt.float16, tag="z",
                                 name=f"z{l}_{mb}")
                    psA = [pz.tile([128, NF], mdt.float32, tag="pz",
                                   name=f"pz{l}_{ma}_{n}") for n in range(NCH)]
                    psB = [pz.tile([128, NF], mdt.float32, tag="pz",
                                   name=f"pz{l}_{mb}_{n}") for n in range(NCH)]
                    group_mms(W8, A8, KT, psA, psB, mcA, mcB, g == 0)
                    for j, (z16, ps) in enumerate(((zA, psA), (zB, psB))):
                        for n in range(NCH):
                            nc.scalar.activation(
                                out=z16[:, n * NF:(n + 1) * NF], in_=ps[n],
                                func=ACT.Copy,
                                accum_out=tpart[:, 4 * j + n:4 * j + n + 1],
                            )
                    nc.gpsimd.dma_start(cc_in[l][g][:, :], tpart)
                    nc.gpsimd.collective_compute(
                        "AllReduce",
                        ALU.add,
                        replica_groups=[list(range(NCORES))],
                        ins=[cc_in[l][g][:, :]],
                        outs=[cc_out[l][g][:, :]],
                    )
                    tg = stat.tile([128, 8], mdt.float32, tag="tg",
                                   bufs=3, name=f"tg{l}_{g}")
                    nc.gpsimd.dma_start(tg, cc_out[l][g][:, :])
                    thr = stat.tile([128, 2], mdt.float32, tag="thr",
                                    bufs=3, name=f"th{l}_{g}")
                    for j in range(2):
                        nc.vector.tensor_reduce(
                            thr[:, j:j + 1], tg[:, 4 * j:4 * j + 4],
                            mybir.AxisListType.X, ALU.add,
                        )
                    nc.vector.tensor_scalar_mul(thr, thr, 1.0 / B)
                    # S = (z >= mean) as {0,1}: BN's mean subtraction cancels
                    # any per-row affine encoding offset downstream, so {0,1}
                    # is as good as +-1 for hidden activations.  tensor_tensor
                    # keeps the fp32 threshold exact (tensor_scalar degrades
                    # AP scalars against 16-bit inputs on HW).
                    for j, z16 in enumerate((zA, zB)):
                        mm = 2 * g + j
                        nc.vector.tensor_tensor(
                            out=S_out[mm // 2][:, mm % 2, :],
                            in0=z16,
                            in1=thr[:, j:j + 1].broadcast_to([128, BL]),
                            op=ALU.is_ge,
                        )
                    # deferred weight binarize after the AR-dependent DVE ops
                    # so a layer-boundary binarize never queues behind them
                    for _ in range(2):
                        if deferred:
                            bin_w(*deferred.pop(0))

            # ---- final layer ----
            # S3 is {0,1}-encoded: psum = sum_k a3*w4~ = (d4 + R4_m)/4 with
            # R4_m = sum_k sign(W4[m,k]).  The true output is
            # out = d4 + sign(b4) = 4*psum - R4_m + sign(b4); R4 = 2*rowsum(w4~)
            # comes from 16 tiny DoubleRow matmuls against a ones vector.
            def final_layer(A8, W8):
                K, H = LAYERS[3]
                KT, MT = K // 128, H // 128
                ones8 = stat.tile([128, 2, 1], FP8, tag="ones")
                nc.vector.memset(ones8, 1.0)
                rps = pz.tile([128, NF], mdt.float32, tag="pz", name="rps")
                for m in range(MT):
                    mc = slice(m * 128, (m + 1) * 128)
                    for kp in range(KT // 2):
                        nc.tensor.matmul(
                            rps[:, m:m + 1], W8[kp][:, :, mc], ones8,
                            start=(kp == 0), stop=(kp == KT // 2 - 1),
                            perf_mode=DR,
                        )
                bias4 = stat.tile([128, MT], mdt.float32, tag="bias4")
                nc.vector.tensor_single_scalar(
                    out=bias4, in_=rps[:, 0:MT], scalar=-2.0, op=ALU.mult,
                )
                nc.vector.tensor_add(bias4, bias4, sb4)
                for g in range(MT // 2):
                    ma, mb = 2 * g, 2 * g + 1
                    mcA = slice(ma * 128, (ma + 1) * 128)
                    mcB = slice(mb * 128, (mb + 1) * 128)
                    psA = [pz.tile([128, NF], mdt.float32, tag="pz",
                                   name=f"pz3_{ma}_{n}") for n in range(NCH)]
                    psB = [pz.tile([128, NF], mdt.float32, tag="pz",
                                   name=f"pz3_{mb}_{n}") for n in range(NCH)]
                    group_mms(W8, A8, KT, psA, psB, mcA, mcB, True)
                    for (ps, m, mc) in ((psA, ma, mcA), (psB, mb, mcB)):
                        for n in range(NCH):
                            oc = ou.tile([128, NF], mdt.float16, tag="ot",
                                         bufs=4, name=f"ot{m}_{n}")
                            nc.scalar.activation(
                                out=oc, in_=ps[n],
                                func=ACT.Identity, bias=bias4[:, m:m + 1],
                                scale=4.0,
                            )
                            nc.sync.dma_start(
                                outT[mc, n * NF:(n + 1) * NF], oc,
                            )

            # layer 1 (W2 binarize interleaved)
            S1 = alloc_s(sB, H1, "s1")
            hidden_layer(0, S0, W8_1, S1, W2_def)

            # layer 2 (W3 prep DMA + deferred binarize interleaved)
            W3_def = []
            W8_3 = []
            for kp in range(H2 // 256):
                halves, w8 = dma_w_tile(wA, 2, kp)
                W3_def.extend(halves)
                W8_3.append(w8)
            S2 = alloc_s(sA, H2, "s2")
            hidden_layer(1, S1, W8_2, S2, W3_def)

            # layer 3 (W4 prep interleaved)
            W4_def = []
            W8_4 = []
            for kp in range(H3 // 256):
                halves, w8 = dma_w_tile(wB, 3, kp)
                W4_def.extend(halves)
                W8_4.append(w8)
            S3 = alloc_s(sB, H3, "s3")
            hidden_layer(2, S2, W8_3, S3, W4_def)

            # layer 4
            final_layer(S3, W8_4)

    nc.compile()
    return nc


def _get_module():
    if "nc" not in _CACHE:
        _CACHE["nc"] = _build_module()
    return _CACHE["nc"]


def _reference_fallback(x, W1, b1, g1, be1, W2, b2, g2, be2, W3, b3, g3, be3, W4, b4):
    """Exact numpy clone of the reference for non-trivial gamma/beta inputs."""
    EPS = 1e-5

    def binarize(v):
        return np.where(v >= 0, 1.0, -1.0).astype(np.float32)

    def bin_linear(a, W, b):
        return binarize(a) @ binarize(W).T + binarize(b)

    def bn(z, g, be):
        m = z.mean(axis=0)
        v = z.var(axis=0)
        return (z - m) / np.sqrt(v + EPS) * g + be

    h = np.clip(bn(bin_linear(x, W1, b1), g1, be1), -1.0, 1.0)
    h = np.clip(bn(bin_linear(h, W2, b2), g2, be2), -1.0, 1.0)
    h = np.clip(bn(bin_linear(h, W3, b3), g3, be3), -1.0, 1.0)
    return bin_linear(h, W4, b4).astype(np.float32)


def make_in_maps(inputs):
    bf16 = ml_dtypes.bfloat16
    x = inputs["x"]
    common = {
        "w1t": np.ascontiguousarray(np.asarray(inputs["W1"]).T).astype(bf16),
        "w2t": np.ascontiguousarray(np.asarray(inputs["W2"]).T).astype(bf16),
        "w3t": np.ascontiguousarray(np.asarray(inputs["W3"]).T).astype(bf16),
        "w4t": np.ascontiguousarray(np.asarray(inputs["W4"]).T).astype(bf16),
        "b4": np.asarray(inputs["b4"], dtype=np.float32).reshape(C, 1),
    }
    in_maps = []
    for c in range(NCORES):
        m = dict(common)
        m["xT"] = np.ascontiguousarray(
            np.asarray(x[c * BL:(c + 1) * BL, :]).T
        ).astype(bf16)
        in_maps.append(m)
    return in_maps


def gather_output(results):
    out = np.empty((B, C), dtype=np.float32)
    for c in range(NCORES):
        out[c * BL:(c + 1) * BL, :] = results[c]["outT"].T.astype(np.float32)
    return out


def kernel(**inputs):
    # BN gamma/beta must be trivial for the sign-reduction; spec fills guarantee
    # this (g=ones, be=zeros).  Anything else falls back to exact host compute.
    for gk, bek in (("g1", "be1"), ("g2", "be2"), ("g3", "be3")):
        if not (np.all(np.asarray(inputs[gk]) == 1.0)
                and np.all(np.asarray(inputs[bek]) == 0.0)):
            return _reference_fallback(**{
                k: np.asarray(v, dtype=np.float32) for k, v in inputs.items()
            })

    from concourse.bass_utils import run_bass_kernel_spmd

    nc = _get_module()
    in_maps = make_in_maps(inputs)
    res = run_bass_kernel_spmd(nc, in_maps, list(range(NCORES)))
    return gather_output(res.results)


if __name__ == "__main__":
    nc = _get_module()
    print("module built OK")


# revision 31
# speedup vs baseline: 1.0353x; 1.0168x over previous
"""Trainium2 Bass kernel for nn_NeuralNetworkSimplified (binarized 4-layer MLP + BN).

Math
----
reference computes, per hidden layer l (gamma=1, beta=0, biases b_l arbitrary):
    z = sign(a) @ sign(W).T + sign(b)
    h = clip(batchnorm_train(z), -1, 1)
and the next layer only consumes sign(h).  Since batchnorm's rsqrt(var+eps) > 0
and gamma=1/beta=0, sign(h) = sign(z - mean_batch(z)); the sign(b) bias shifts
z and its mean equally, so it cancels.  The whole network therefore reduces to
exact integer arithmetic:
    S0 = sign(x)
    S_l = sign(Z_l - colmean(Z_l)),   Z_l = S_{l-1} @ sign(W_l).T
    out = S3 @ sign(W4).T + sign(b4)

On device activations are +-0.5 (fp8) and weights +-1 (fp8), so Z comes out as
exact half-scaled integer dot products (|Z| <= 1536: exact in fp32 psum and in
the fp16 spill).  The batch mean needs a cross-core reduction: each core
computes the column sum of its local Z block as a fused by-product of the
PSUM->SBUF spill on the Scalar engine (activation Copy with accum_out), then
AllReduces the tiny [128, 8] partial-sum tile per pair of output blocks.
Binarize (DVE) reads the fp16 Z copy, so PSUM recycles immediately and the
AllReduce latency never blocks the PE.

Sharding: batch 16384 -> 8 cores x 2048.  Activations live feature-major
(transposed) so the contraction dim sits on SBUF partitions; host passes x.T
shards and W_l.T (layout-only prep; all FLOPs incl. sign() run on device).

Engines: PE runs only the z-matmuls (fp8 DoubleRow).  Scalar does all weight
binarize (Sign) + Z spills + final bias.  DVE/GpSimd split the x binarize; DVE
does S binarize.  DMA of x/W1 is interleaved kp-by-kp so layer 1 streams
right behind the load.
"""

import numpy as np
import ml_dtypes

B, D, H1, H2, H3, C = 16384, 3072, 2048, 2048, 1024, 512
NCORES = 8
BL = B // NCORES          # 2048 rows per core
NF = 512                  # batch free-dim chunk (psum tile width)
NCH = BL // NF            # 4 chunks
LAYERS = [(D, H1), (H1, H2), (H2, H3), (H3, C)]

_CACHE = {}


def _build_module():
    import concourse.bass as bass
    import concourse.mybir as mybir
    import concourse.tile as tile
    from concourse import bacc

    mdt = mybir.dt
    FP8 = mdt.float8e4
    ALU = mybir.AluOpType
    ACT = mybir.ActivationFunctionType
    DR = mybir.MatmulPerfMode.DoubleRow

    nc = bacc.Bacc(
        "TRN2",
        target_bir_lowering=False,
        debug=False,
        num_devices=NCORES,
    )

    xT = nc.dram_tensor("xT", [D, BL], mdt.bfloat16, kind="ExternalInput").ap()
    wT = [
        nc.dram_tensor(f"w{i + 1}t", [K, H], mdt.bfloat16, kind="ExternalInput").ap()
        for i, (K, H) in enumerate(LAYERS)
    ]
    b4 = nc.dram_tensor("b4", [C, 1], mdt.float32, kind="ExternalInput").ap()
    outT = nc.dram_tensor("outT", [C, BL], mdt.float16, kind="ExternalOutput").ap()

    # t AllReduce buffers: one per (hidden layer, 2-block group); each carries
    # 4 chunk-partials x 2 blocks = [128, 8] fp32.
    n_groups = [H1 // 256, H2 // 256, H3 // 256]   # 8, 8, 4
    cc_in = [
        [nc.dram_tensor(f"tin{l}_{g}", [128, 8], mdt.float32).ap()
         for g in range(n_groups[l])]
        for l in range(3)
    ]
    cc_out = [
        [nc.dram_tensor(f"tout{l}_{g}", [128, 8], mdt.float32,
                        addr_space="Shared").ap()
         for g in range(n_groups[l])]
        for l in range(3)
    ]

    with tile.TileContext(nc, num_cores=NCORES) as tc:
        with (
            tc.tile_pool(name="rawx", bufs=2) as rawx,     # x staging pair tiles
            tc.tile_pool(name="raww", bufs=2) as raww,     # W staging pair tiles
            tc.tile_pool(name="sA", bufs=12) as sA,        # S0, S2 pair tiles
            tc.tile_pool(name="sB", bufs=8) as sB,         # S1, S3 pair tiles
            tc.tile_pool(name="wA", bufs=12) as wA,        # W1~, W3~ pair tiles
            tc.tile_pool(name="wB", bufs=8) as wB,         # W2~, W4~ pair tiles
            tc.tile_pool(name="zf", bufs=5) as zf,         # fp16 Z spill blocks
            tc.tile_pool(name="stat", bufs=2) as stat,     # partial sums etc.
            tc.tile_pool(name="ou", bufs=4) as ou,         # fp16 output chunks
            tc.tile_pool(name="pz", bufs=8, space="PSUM") as pz,
        ):
            # ---- PE warm-up: dummy matmuls so HAM unthrottles while the
            # binarize prologue runs on the other engines.
            warm = stat.tile([128, 128], FP8, tag="warm")
            nc.vector.memset(warm, 0.5)
            wps = pz.tile([128, NF], mdt.float32, tag="pz", name="warmps")
            for i in range(24):
                nc.tensor.matmul(wps[:, 0:128], warm, warm, start=True, stop=True)

            # ---- sign(b4) as +-1 per-partition vector, [128, C//128] ----
            b4_sb = stat.tile([128, C // 128], mdt.float32, tag="rawb")
            nc.sync.dma_start(b4_sb, b4.rearrange("(o p) q -> p (o q)", p=128))
            sb4 = stat.tile([128, C // 128], mdt.float32, tag="sb4")
            nc.vector.tensor_scalar(
                out=sb4, in0=b4_sb, scalar1=0.0, scalar2=2.0,
                op0=ALU.is_ge, op1=ALU.mult,
            )
            nc.vector.tensor_scalar_add(sb4, sb4, -1.0)

            def bin_w(w8, rt):
                # sign(W)/2 as +-0.5 fp8 on DVE.  The two scalar literals
                # (0.0, 0.5) are exact in every intermediate precision, so
                # this is immune to the tensor_scalar scalar-degrade issue.
                nc.vector.tensor_scalar(
                    out=w8, in0=rt, scalar1=0.0, scalar2=0.5,
                    op0=ALU.is_ge, op1=ALU.subtract,
                )

            def dma_w_tile(pool, idx, kp, dma_engs=None):
                # DMA the two k-halves of a weight pair tile; binarize is a
                # deferred list of per-half (dst, src) DVE ops.
                K, H = LAYERS[idx]
                w8 = pool.tile([128, 2, H], FP8, tag="w", name=f"w{idx}_{kp}")
                halves = []
                for h in range(2):
                    rt = raww.tile([128, H], mdt.bfloat16, tag="raww",
                                   name=f"rw{idx}_{kp}_{h}")
                    eng = dma_engs[h] if dma_engs else nc.sync
                    eng.dma_start(
                        rt,
                        wT[idx][(2 * kp + h) * 128:(2 * kp + h + 1) * 128, :],
                    )
                    halves.append((w8[:, h, :], rt))
                return halves, w8

            # ---- x + W1 prologue, interleaved kp-by-kp on the sync queue so
            # layer 1's contraction streams right behind the load.
            S0 = []
            W8_1 = []
            for kp in range(D // 256):
                s8 = sA.tile([128, 2, BL], FP8, tag="s", name=f"s0_{kp}")
                for h in range(2):
                    rx = rawx.tile([128, BL], mdt.bfloat16, tag="rawx",
                                   name=f"rx{kp}_{h}")
                    # split the stream over the two HWDGE queues (sync + ACT):
                    # one queue serializes 512KB transfers at ~240 GB/s; two
                    # run at the HBM cap.
                    (nc.sync if h == 0 else nc.scalar).dma_start(
                        rx,
                        xT[(2 * kp + h) * 128:(2 * kp + h + 1) * 128, :],
                    )
                    # sign(x) as +-1 fp8 via ACT (runs parallel to the DVE
                    # weight binarize during the DMA-bound startup).  x is
                    # never exactly +-0.0 (bf16 keeps fp32's exponent range),
                    # so Sign(0)=0 cannot occur.
                    nc.scalar.activation(out=s8[:, h, :], in_=rx, func=ACT.Sign)
                S0.append(s8)
                halves, w8 = dma_w_tile(wA, 0, kp,
                                        dma_engs=(nc.sync, nc.scalar))
                for dst, src in halves:
                    bin_w(dst, src)
                W8_1.append(w8)

            # W2 raw DMAs issue now (sync queue, after x/W1); binarize is
            # deferred into the layer-1 m-loop so Scalar prioritizes spills.
            W2_def = []
            W8_2 = []
            for kp in range(H1 // 256):
                halves, w8 = dma_w_tile(wB, 1, kp)
                W2_def.extend(halves)
                W8_2.append(w8)

            def alloc_s(pool, H, nm):
                return [
                    pool.tile([128, 2, BL], FP8, tag="s", name=f"{nm}_{i}")
                    for i in range(H // 256)
                ]

            # ---- shared matmul emission for a 2-block group ----
            # For g == 0 the two blocks' contraction loops are interleaved
            # (all-but-last kp for both, then last kp for both): the last kp
            # consumes the previous layer's final S pair, which waits on that
            # layer's last AllReduce, so this buys ~2 blocks of PE cover.
            # Also streams layer 1 behind the x/W1 DMA arrival order.
            def group_mms(W8, A8, KT, psA, psB, mcA, mcB, interleave):
                KH = KT // 2

                def mm(ps, mc, kp):
                    st, sp = kp == 0, kp == KH - 1
                    wsl = W8[kp][:, :, mc]
                    for n in range(NCH):
                        nc.tensor.matmul(
                            ps[n], wsl, A8[kp][:, :, n * NF:(n + 1) * NF],
                            start=st, stop=sp, perf_mode=DR,
                        )

                if interleave:
                    for kp in range(KH - 1):
                        mm(psA, mcA, kp)
                    for kp in range(KH - 1):
                        mm(psB, mcB, kp)
                    mm(psA, mcA, KH - 1)
                    mm(psB, mcB, KH - 1)
                else:
                    for kp in range(KH):
                        mm(psA, mcA, kp)
                    for kp in range(KH):
                        mm(psB, mcB, kp)

            # ---- one hidden layer ----
            # Per output block m: z-matmuls -> 4 psum chunks; Scalar spills
            # each chunk into the fp16 Z tile with a fused column-sum partial;
            # per 2-block group AllReduce the [128, 8] partials; DVE reduces
            # the 4 global partials, scales by 1/B, binarizes Z vs threshold.
            def hidden_layer(l, A8, W8, S_out, deferred):
                K, H = LAYERS[l]
                KT, MT = K // 128, H // 128
                for g in range(MT // 2):
                    ma, mb = 2 * g, 2 * g + 1
                    mcA = slice(ma * 128, (ma + 1) * 128)
                    mcB = slice(mb * 128, (mb + 1) * 128)
                    tpart = stat.tile([128, 8], mdt.float32, tag="tpart",
                                      bufs=4, name=f"tp{l}_{g}")
                    zA = zf.tile([128, BL], mdt.float16, tag="z",
                                 name=f"z{l}_{ma}")
                    zB = zf.tile([128, BL], mdt.float16, tag="z",
                                 name=f"z{l}_{mb}")
                    psA = [pz.tile([128, NF], mdt.float32, tag="pz",
                                   name=f"pz{l}_{ma}_{n}") for n in range(NCH)]
                    psB = [pz.tile([128, NF], mdt.float32, tag="pz",
                                   name=f"pz{l}_{mb}_{n}") for n in range(NCH)]
                    group_mms(W8, A8, KT, psA, psB, mcA, mcB, g == 0)
                    for j, (z16, ps) in enumerate(((zA, psA), (zB, psB))):
                        for n in range(NCH):
                            nc.scalar.activation(
                                out=z16[:, n * NF:(n + 1) * NF], in_=ps[n],
                                func=ACT.Copy,
                                accum_out=tpart[:, 4 * j + n:4 * j + n + 1],
                            )
                    # deferred weight binarize right after the spills: these
                    # must never queue behind AR-gated ops, or the next
                    # layer's matmuls stall on unbinarized weights
                    for _ in range(2):
                        if deferred:
                            bin_w(*deferred.pop(0))
                    nc.gpsimd.dma_start(cc_in[l][g][:, :], tpart)
                    nc.gpsimd.collective_compute(
                        "AllReduce",
                        ALU.add,
                        replica_groups=[list(range(NCORES))],
                        ins=[cc_in[l][g][:, :]],
                        outs=[cc_out[l][g][:, :]],
                    )
                    tg = stat.tile([128, 8], mdt.float32, tag="tg",
                                   bufs=3, name=f"tg{l}_{g}")
                    nc.gpsimd.dma_start(tg, cc_out[l][g][:, :])
                    thr = stat.tile([128, 2], mdt.float32, tag="thr",
                                    bufs=3, name=f"th{l}_{g}")
                    for j in range(2):
                        nc.vector.tensor_reduce(
                            thr[:, j:j + 1], tg[:, 4 * j:4 * j + 4],
                            mybir.AxisListType.X, ALU.add,
                        )
                    nc.vector.tensor_scalar_mul(thr, thr, 1.0 / B)
                    # S = (z >= mean) as {0,1}: BN's mean subtraction cancels
                    # any per-row affine encoding offset downstream, so {0,1}
                    # is as good as +-1 for hidden activations.  tensor_tensor
                    # keeps the fp32 threshold exact (tensor_scalar degrades
                    # AP scalars against 16-bit inputs on HW).  Chunked and
                    # interleaved across the pair so the next layer's matmuls
                    # (which consume both blocks chunk-by-chunk) start sooner
                    # after the AllReduce lands.
                    for n in range(NCH):
                        for j, z16 in enumerate((zA, zB)):
                            mm = 2 * g + j
                            csl = slice(n * NF, (n + 1) * NF)
                            nc.vector.tensor_tensor(
                                out=S_out[mm // 2][:, mm % 2, csl],
                                in0=z16[:, csl],
                                in1=thr[:, j:j + 1].broadcast_to([128, NF]),
                                op=ALU.is_ge,
                            )

            # ---- final layer ----
            # S3 is {0,1}-encoded: psum = sum_k a3*w4~ = (d4 + R4_m)/4 with
            # R4_m = sum_k sign(W4[m,k]).  The true output is
            # out = d4 + sign(b4) = 4*psum - R4_m + sign(b4); R4 = 2*rowsum(w4~)
            # comes from 16 tiny DoubleRow matmuls against a ones vector.
            def final_layer(A8, W8):
                K, H = LAYERS[3]
                KT, MT = K // 128, H // 128
                ones8 = stat.tile([128, 2, 1], FP8, tag="ones")
                nc.vector.memset(ones8, 1.0)
                rps = pz.tile([128, NF], mdt.float32, tag="pz", name="rps")
                for m in range(MT):
                    mc = slice(m * 128, (m + 1) * 128)
                    for kp in range(KT // 2):
                        nc.tensor.matmul(
                            rps[:, m:m + 1], W8[kp][:, :, mc], ones8,
                            start=(kp == 0), stop=(kp == KT // 2 - 1),
                            perf_mode=DR,
                        )
                bias4 = stat.tile([128, MT], mdt.float32, tag="bias4")
                nc.vector.tensor_single_scalar(
                    out=bias4, in_=rps[:, 0:MT], scalar=-2.0, op=ALU.mult,
                )
                nc.vector.tensor_add(bias4, bias4, sb4)
                for g in range(MT // 2):
                    ma, mb = 2 * g, 2 * g + 1
                    mcA = slice(ma * 128, (ma + 1) * 128)
                    mcB = slice(mb * 128, (mb + 1) * 128)
                    psA = [pz.tile([128, NF], mdt.float32, tag="pz",
                                   name=f"pz3_{ma}_{n}") for n in range(NCH)]
                    psB = [pz.tile([128, NF], mdt.float32, tag="pz",
                                   name=f"pz3_{mb}_{n}") for n in range(NCH)]
                    group_mms(W8, A8, KT, psA, psB, mcA, mcB, True)
                    for (ps, m, mc) in ((psA, ma, mcA), (psB, mb, mcB)):
                        for n in range(NCH):
                            oc = ou.tile([128, NF], mdt.float16, tag="ot",
                                         bufs=4, name=f"ot{m}_{n}")
                            nc.scalar.activation(
                                out=oc, in_=ps[n],
                                func=ACT.Identity, bias=bias4[:, m:m + 1],
                                scale=4.0,
                            )
                            nc.sync.dma_start(
                                outT[mc, n * NF:(n + 1) * NF], oc,
                            )

            # layer 1 (W2 binarize interleaved)
            S1 = alloc_s(sB, H1, "s1")
            hidden_layer(0, S0, W8_1, S1, W2_def)

            # layer 2 (W3 prep DMA + deferred binarize interleaved)
            W3_def = []
            W8_3 = []
            for kp in range(H2 // 256):
                halves, w8 = dma_w_tile(wA, 2, kp)
                W3_def.extend(halves)
                W8_3.append(w8)
            S2 = alloc_s(sA, H2, "s2")
            hidden_layer(1, S1, W8_2, S2, W3_def)

            # layer 3 (W4 prep interleaved)
            W4_def = []
            W8_4 = []
            for kp in range(H3 // 256):
                halves, w8 = dma_w_tile(wB, 3, kp)
                W4_def.extend(halves)
                W8_4.append(w8)
            S3 = alloc_s(sB, H3, "s3")
            hidden_layer(2, S2, W8_3, S3, W4_def)

            # layer 4
            final_layer(S3, W8_4)

    nc.compile()
    return nc


def _get_module():
    if "nc" not in _CACHE:
        _CACHE["nc"] = _build_module()
    return _CACHE["nc"]


def _reference_fallback(x, W1, b1, g1, be1, W2, b2, g2, be2, W3, b3, g3, be3, W4, b4):
    """Exact numpy clone of the reference for non-trivial gamma/beta inputs."""
    EPS = 1e-5

    def binarize(v):
        return np.where(v >= 0, 1.0, -1.0).astype(np.float32)

    def bin_linear(a, W, b):
        return binarize(a) @ binarize(W).T + binarize(b)

    def bn(z, g, be):
        m = z.mean(axis=0)
        v = z.var(axis=0)
        return (z - m) / np.sqrt(v + EPS) * g + be

    h = np.clip(bn(bin_linear(x, W1, b1), g1, be1), -1.0, 1.0)
    h = np.clip(bn(bin_linear(h, W2, b2), g2, be2), -1.0, 1.0)
    h = np.clip(bn(bin_linear(h, W3, b3), g3, be3), -1.0, 1.0)
    return bin_linear(h, W4, b4).astype(np.float32)


def make_in_maps(inputs):
    bf16 = ml_dtypes.bfloat16
    x = inputs["x"]
    common = {
        "w1t": np.ascontiguousarray(np.asarray(inputs["W1"]).T).astype(bf16),
        "w2t": np.ascontiguousarray(np.asarray(inputs["W2"]).T).astype(bf16),
        "w3t": np.ascontiguousarray(np.asarray(inputs["W3"]).T).astype(bf16),
        "w4t": np.ascontiguousarray(np.asarray(inputs["W4"]).T).astype(bf16),
        "b4": np.asarray(inputs["b4"], dtype=np.float32).reshape(C, 1),
    }
    in_maps = []
    for c in range(NCORES):
        m = dict(common)
        m["xT"] = np.ascontiguousarray(
            np.asarray(x[c * BL:(c + 1) * BL, :]).T
        ).astype(bf16)
        in_maps.append(m)
    return in_maps


def gather_output(results):
    out = np.empty((B, C), dtype=np.float32)
    for c in range(NCORES):
        out[c * BL:(c + 1) * BL, :] = results[c]["outT"].T.astype(np.float32)
    return out


def kernel(**inputs):
    # BN gamma/beta must be trivial for the sign-reduction; spec fills guarantee
    # this (g=ones, be=zeros).  Anything else falls back to exact host compute.
    for gk, bek in (("g1", "be1"), ("g2", "be2"), ("g3", "be3")):
        if not (np.all(np.asarray(inputs[gk]) == 1.0)
                and np.all(np.asarray(inputs[bek]) == 0.0)):
            return _reference_fallback(**{
                k: np.asarray(v, dtype=np.float32) for k, v in inputs.items()
            })

    from concourse.bass_utils import run_bass_kernel_spmd

    nc = _get_module()
    in_maps = make_in_maps(inputs)
    res = run_bass_kernel_spmd(nc, in_maps, list(range(NCORES)))
    return gather_output(res.results)


if __name__ == "__main__":
    nc = _get_module()
    print("module built OK")
